# revision 65
# baseline (speedup 1.0000x reference)
"""Trainium2 8-core Bass kernel for a dense transformer block.

Strategy (Megatron-style hybrid):
  - LN1 sequence-parallel (each core norms its 512-token slice), PE-transpose,
    AllGather -> hT (full [C, N] transposed activations, bf16, on every core).
  - QKV tensor-parallel over heads: each core computes qT/kT/vT for its 2
    heads over all 4096 tokens (contraction over C in 128-chunks, bf16).
  - Causal attention per (batch, local-head) in transposed layout:
    sT = K @ qT (keys on partitions), exp with no max subtraction (scores are
    O(few) for LN'd inputs; verified against the reference distribution),
    y'T = [V | 1]^T @ exp(sT) accumulated over key tiles; row 64 of y'T is
    the softmax denominator. PE-transpose + per-partition reciprocal gives y
    in [token, d] layout.
  - y exchanged via AllToAll (channel-shard -> token-shard).
  - x2 = x + y, LN2, MLP data-parallel over the core's 512 tokens with full
    bf16 weights streamed from HBM; out = x2 (+ b_proj) + mlp.
Outputs per core: out_x [512, C] token slice; out_k/out_v [B, 2, T, D] for its
two heads. Host assembles the full (x, k, v) tuple.
"""

import os
import sys
from contextlib import ExitStack

for _p in ("/root/.axon_site/_ro/trn_rl_repo", "/opt/trn_rl_repo"):
    if os.path.isdir(_p) and _p not in sys.path:
        sys.path.append(_p)

import numpy as np
import ml_dtypes

import concourse.bass as bass
import concourse.tile as tile
from concourse import mybir
from concourse.bass_utils import run_bass_kernel_spmd
from concourse.masks import make_identity
import bass_rust

F32 = mybir.dt.float32
BF16 = mybir.dt.bfloat16

NCORES = 8
B, T, C = 2, 2048, 1024
H, D = 16, 64
F = 4 * C                    # 4096
N = B * T                    # 4096 flat tokens
TOK = N // NCORES            # 512 tokens per core
HL = H // NCORES             # 2 heads per core
EPS = 1e-5
NEG = -30000.0

KT = T // 128                # 16 key tiles per batch
QC = T // 512                # 4 query chunks of 512 per batch
NTILE = TOK // 128           # 4 token tiles per core


def _split_multi_waits(nc):
    """walrus here rejects >1 sem-wait per instruction; split extras into
    standalone single-wait EventSemaphore ops on the same engine."""
    ctr = 0
    for f in nc.m.functions:
        for bb in f.blocks:
            insts = bb.instructions
            i = 0
            while i < len(insts):
                ins = insts[i]
                si = getattr(ins, "sync_info", None)
                if si is not None and len(si.on_wait) > 1:
                    waits = list(si.on_wait)
                    new_ops = []
                    for w in waits[:-1]:
                        ctr += 1
                        es = mybir.InstEventSemaphore(
                            name=f"WSPLIT-{ctr}", ins=[], outs=[])
                        es.engine = ins.engine
                        es.sync_info = bass_rust.SyncInfo(
                            on_wait=[w], on_update=[])
                        new_ops.append(es)
                    ins.sync_info = bass_rust.SyncInfo(
                        on_wait=[waits[-1]], on_update=list(si.on_update))
                    insts[i:i] = new_ops
                    i += len(new_ops)
                i += 1


def _emit(nc, tc, io, ctx, trivial):
    """trivial=True: ln weights are ones, all biases zero (skip affine/bias)."""
    RG = [list(range(NCORES))]
    dma = nc.sync.dma_start
    AF = mybir.ActivationFunctionType
    OP = mybir.AluOpType

    const = ctx.enter_context(tc.tile_pool(name="const", bufs=1))
    work = ctx.enter_context(tc.tile_pool(name="work", bufs=3))
    big = ctx.enter_context(tc.tile_pool(name="big", bufs=2))
    trp = ctx.enter_context(tc.tile_pool(name="trp", bufs=4))
    psum = ctx.enter_context(tc.tile_pool(name="psum", bufs=1, space="PSUM"))
    dram = ctx.enter_context(tc.tile_pool(name="dram", bufs=1, space="DRAM"))

    # ---------------- constants ----------------
    ident_bf = const.tile([128, 128], BF16)
    make_identity(nc, ident_bf[:])
    ident_f = const.tile([128, 128], F32)
    make_identity(nc, ident_f[:])

    eps_t = const.tile([128, 1], F32)
    nc.vector.memset(eps_t[:], EPS)

    # causal multiplicative base: base[p, m] = 1 if m >= p + 512 else 0.
    # For a diagonal key-tile at query offset jl, slice [:, 512-jl : 1024-jl];
    # applied to exp(scores), since exp of an unmasked score can't overflow
    # here (LN'd inputs keep scores O(10)).
    causal = const.tile([128, 1024], BF16)
    nc.gpsimd.memset(causal[:], 1.0)
    nc.gpsimd.affine_select(
        out=causal[:], in_=causal[:], compare_op=OP.is_ge,
        fill=0.0, base=-512, channel_multiplier=-1, pattern=[[1, 1024]])

    if not trivial:
        def bc_row(name, ap_src):
            # bf16 to keep the fallback path within SBUF; mixed-dtype DVE
            # tensor_tensor handles the f32 operand side
            t = const.tile([128, C], BF16, name=name)
            nc.gpsimd.dma_start(out=t[:],
                                in_=ap_src.partition_broadcast(128).opt())
            return t
        ln1w_bc = bc_row("ln1w_bc", io["ln1_w"])
        ln1b_bc = bc_row("ln1b_bc", io["ln1_b"])
        ln2w_bc = bc_row("ln2w_bc", io["ln2_w"])
        ln2b_bc = bc_row("ln2b_bc", io["ln2_b"])
        bproj_bc = bc_row("bproj_bc", io["b_proj"])
        bq_sb = const.tile([128, 1], F32)
        dma(out=bq_sb[:], in_=io["bq_s"][:])
        bk_sb = const.tile([128, 1], F32)
        dma(out=bk_sb[:], in_=io["bk_s"][:])
        bv_sb = const.tile([128, 1], F32)
        dma(out=bv_sb[:], in_=io["bv_s"][:])
        bfc_sb = const.tile([128, F // 128], F32)
        dma(out=bfc_sb[:], in_=io["b_fc_s"][:])

    # QKV weight slices, [C, 128] viewed as [128(c), 8(cc), 128(ch)], bf16
    wq_sb = const.tile([128, 8, 128], BF16)
    dma(out=wq_sb[:], in_=io["wq_s"].rearrange("(cc p) h -> p cc h", p=128))
    wk_sb = const.tile([128, 8, 128], BF16)
    dma(out=wk_sb[:], in_=io["wk_s"].rearrange("(cc p) h -> p cc h", p=128))
    wv_sb = const.tile([128, 8, 128], BF16)
    dma(out=wv_sb[:], in_=io["wv_s"].rearrange("(cc p) h -> p cc h", p=128))

    # persistent activations
    qT = const.tile([128, N], BF16)   # [2 heads x 64, all tokens]
    kT = const.tile([128, N], BF16)
    vT = const.tile([128, N], BF16)
    h2T = const.tile([128, 8, TOK], BF16)
    gT = const.tile([128, F // 128, TOK], BF16)

    # DRAM internal + collective buffers (h travels in natural layout; the
    # transpose happens in the DMA xbar on the way back into SBUF)
    h_loc = dram.tile([TOK, C], BF16)
    h_all = dram.tile([NCORES * TOK, C], BF16,
                      addr_space="Shared" if nc.num_devices > 1 else "Local")
    y_loc = dram.tile([N, HL * D], F32)
    y_a2a = dram.tile([N, HL * D], F32)

    x2_pool = ctx.enter_context(tc.tile_pool(name="x2", bufs=NTILE))
    x2_tiles = []

    # =============== LN1 (own 512 tokens) + transpose -> hT_loc ===========
    def layernorm(x_t, w_bc, b_bc, out_bf):
        """out_bf = LN(x_t) [* w + b] ; x_t [128, C] f32, out bf16."""
        stats = work.tile([128, 2, 6], F32, tag="ln_stats")
        xr = x_t[:].rearrange("p (s f) -> p s f", f=512)
        for s in range(2):
            nc.vector.bn_stats(out=stats[:, s, :], in_=xr[:, s, :])
        mv = work.tile([128, 2], F32, tag="ln_mv")
        nc.vector.bn_aggr(out=mv[:], in_=stats[:])
        sd = work.tile([128, 1], F32, tag="ln_sd")
        nc.scalar.activation(out=sd[:], in_=mv[:, 1:2], func=AF.Sqrt,
                             bias=eps_t[:], scale=1.0)
        rstd = work.tile([128, 1], F32, tag="ln_rstd")
        nc.vector.reciprocal(out=rstd[:], in_=sd[:])
        if trivial:
            nc.vector.tensor_scalar(out=out_bf[:], in0=x_t[:],
                                    scalar1=mv[:, 0:1], scalar2=rstd[:],
                                    op0=OP.subtract, op1=OP.mult)
        else:
            tmp = work.tile([128, C], F32, tag="ln_tmp", bufs=1)
            nc.vector.tensor_scalar(out=tmp[:], in0=x_t[:],
                                    scalar1=mv[:, 0:1], scalar2=rstd[:],
                                    op0=OP.subtract, op1=OP.mult)
            nc.vector.tensor_tensor(out=tmp[:], in0=tmp[:], in1=w_bc[:],
                                    op=OP.mult)
            nc.vector.tensor_tensor(out=out_bf[:], in0=tmp[:], in1=b_bc[:],
                                    op=OP.add)

    for ti in range(NTILE):
        x_t = x2_pool.tile([128, C], F32, tag="x2", name=f"x2_{ti}")
        x2_tiles.append(x_t)
        dma(out=x_t[:], in_=io["x_s"][ti * 128:(ti + 1) * 128, :])
        h_bf = work.tile([128, C], BF16, tag="h1",
                         bufs=3 if trivial else 2)
        layernorm(x_t, ln1w_bc if not trivial else None,
                  ln1b_bc if not trivial else None, h_bf)
        dma(out=h_loc[ti * 128:(ti + 1) * 128, :], in_=h_bf[:])

    if nc.num_devices == 1:  # timing-sim variant: fake the collective
        for rr in range(NCORES):
            dma(out=h_all[rr * TOK:(rr + 1) * TOK, :], in_=h_loc[:])
    else:
        nc.gpsimd.collective_compute(
            "AllGather", mybir.AluOpType.bypass, replica_groups=RG,
            ins=[h_loc.opt()], outs=[h_all.opt()])

    # =============== QKV: qT/kT/vT [128ch, 4096tok] bf16 ==================
    def qkv_chunk(tcn):
        # hT chunk via xbar transpose: [512 tok, 1024 c] -> [128, 8cc, 512]
        rhs = work.tile([128, 8, 512], BF16, tag="hT_rhs", bufs=2,
                        name=f"rhs_{tcn}")
        nc.scalar.dma_start_transpose(
            rhs[:], h_all[tcn * TOK:(tcn + 1) * TOK, :])
        ps_q = psum.tile([128, 512], F32, tag="acc", bufs=3)
        ps_k = psum.tile([128, 512], F32, tag="acc", bufs=3)
        ps_v = psum.tile([128, 512], F32, tag="acc", bufs=3)
        for cc in range(8):
            st, sp = (cc == 0), (cc == 7)
            nc.tensor.matmul(ps_q[:], wq_sb[:, cc, :], rhs[:, cc, :],
                             start=st, stop=sp)
            nc.tensor.matmul(ps_k[:], wk_sb[:, cc, :], rhs[:, cc, :],
                             start=st, stop=sp)
            nc.tensor.matmul(ps_v[:], wv_sb[:, cc, :], rhs[:, cc, :],
                             start=st, stop=sp)
        sl = slice(tcn * 512, (tcn + 1) * 512)
        if trivial:
            nc.vector.tensor_copy(out=qT[:, sl], in_=ps_q[:])
            nc.vector.tensor_copy(out=kT[:, sl], in_=ps_k[:])
            nc.vector.tensor_copy(out=vT[:, sl], in_=ps_v[:])
        else:
            nc.vector.tensor_scalar_add(qT[:, sl], ps_q[:], bq_sb[:])
            nc.vector.tensor_scalar_add(kT[:, sl], ps_k[:], bk_sb[:])
            nc.vector.tensor_scalar_add(vT[:, sl], ps_v[:], bv_sb[:])

    # =============== attention per (batch, local head) ====================
    def attention_batch(b):
        col0 = b * T
        for hl in range(HL):
            ho = hl * D
            # v in natural layout + ones column, via one xbar transpose:
            # [64, 2048] -> [128p x 16kt, 64]
            vn = big.tile([128, KT, D], BF16, tag="vnat")
            nc.scalar.dma_start_transpose(
                vn[:], vT[ho:ho + D, col0:col0 + T])
            vo = big.tile([128, KT, D + 8], BF16, tag="vones")
            nc.vector.memset(vo[:, :, D:D + 1], 1.0)
            nc.vector.tensor_copy(out=vo[:, :, 0:D], in_=vn[:])
            kn = big.tile([128, KT, D], BF16, tag="knat")
            nc.scalar.dma_start_transpose(
                kn[:], kT[ho:ho + D, col0:col0 + T])
            dma(out=io["out_v"][b, hl].rearrange("(kt p) d -> p kt d", p=128),
                in_=vn[:])
            dma(out=io["out_k"][b, hl].rearrange("(kt p) d -> p kt d", p=128),
                in_=kn[:])
            y_st = big.tile([128, KT, D], F32, tag="y_st")
            for qc in range(QC):
                q0 = qc * 512
                nkt = (q0 + 512) // 128
                ps_y = psum.tile([128, 512], F32, tag="acc", bufs=3)
                for kt in range(nkt):
                    k0 = kt * 128
                    # columns below jl are fully masked: skip them.
                    # (jl == 0 for the first kt of every qc, so the start=True
                    # AV matmul always initializes the full PSUM width.)
                    jl = max(0, k0 - q0)
                    w = 512 - jl
                    ps_s = psum.tile([128, 512], F32, tag="mm", bufs=3)
                    nc.tensor.matmul(
                        ps_s[:, 0:w],
                        kT[ho:ho + D, col0 + k0:col0 + k0 + 128],
                        qT[ho:ho + D,
                           col0 + q0 + jl:col0 + q0 + 512],
                        start=True, stop=True)
                    sb_s = work.tile([128, 512], BF16, tag="sb_s",
                                     bufs=6 if trivial else 4)
                    nc.scalar.activation(out=sb_s[:, 0:w], in_=ps_s[:, 0:w],
                                         func=AF.Exp)
                    if k0 >= q0:
                        nc.vector.tensor_tensor(
                            out=sb_s[:, 0:w], in0=sb_s[:, 0:w],
                            in1=causal[:, 512:1024 - jl],
                            op=OP.mult)
                    nc.tensor.matmul(ps_y[0:D + 1, jl:512],
                                     vo[:, kt, 0:D + 1], sb_s[:, 0:w],
                                     start=(kt == 0), stop=(kt == nkt - 1))
                sb_y = work.tile([128, 512], F32, tag="sb_y")
                nc.vector.tensor_copy(out=sb_y[0:D + 1, :],
                                      in_=ps_y[0:D + 1, :])
                for jb in range(4):
                    j0 = jb * 128
                    ps_t = psum.tile([128, D + 1], F32, tag="tr", bufs=2)
                    nc.tensor.transpose(ps_t[:, 0:D + 1],
                                        sb_y[0:D + 1, j0:j0 + 128],
                                        ident_f[0:D + 1, 0:D + 1])
                    sb_t = trp.tile([128, D + 1], F32, tag="sb_t")
                    nc.vector.tensor_copy(out=sb_t[:], in_=ps_t[:, 0:D + 1])
                    rec = trp.tile([128, 1], F32, tag="rec")
                    nc.vector.reciprocal(out=rec[:], in_=sb_t[:, D:D + 1])
                    nc.vector.tensor_scalar_mul(
                        y_st[:, qc * 4 + jb, :], sb_t[:, 0:D], rec[:])
            dma(out=y_loc[col0:col0 + T, ho:ho + D]
                .rearrange("(blk p) d -> p blk d", p=128), in_=y_st[:])

    for tcn in range(8):
        qkv_chunk(tcn)
    for b in range(B):
        attention_batch(b)

    if nc.num_devices == 1:  # timing-sim variant: fake the collective
        dma(out=y_a2a[:], in_=y_loc[:])
    else:
        nc.gpsimd.collective_compute(
            "AllToAll", mybir.AluOpType.bypass, replica_groups=RG,
            ins=[y_loc.opt()], outs=[y_a2a.opt()])

    # =============== x2 = x + y ; LN2 ; h2T ===============================
    for ti in range(NTILE):
        x_t = x2_tiles[ti]
        ytmp = work.tile([128, C], F32, tag="ytmp",
                         bufs=2 if trivial else 1)
        dma(out=ytmp[:].rearrange("p (cr d) -> p cr d", d=HL * D),
            in_=y_a2a[:].rearrange("(cr blk p) d -> p cr blk d",
                                   p=128, blk=NTILE)[:, :, ti, :])
        nc.vector.tensor_tensor(out=x_t[:], in0=x_t[:], in1=ytmp[:],
                                op=OP.add)
        h2_bf = work.tile([128, C], BF16, tag="h2",
                          bufs=3 if trivial else 2)
        layernorm(x_t, ln2w_bc if not trivial else None,
                  ln2b_bc if not trivial else None, h2_bf)
        if not trivial:
            # x2 += b_proj now that LN2 consumed x2
            nc.vector.tensor_tensor(out=x_t[:], in0=x_t[:], in1=bproj_bc[:],
                                    op=OP.add)
        # [128 tok, 1024 c] -> [128p x 8cc, 128 tok] via xbar
        nc.scalar.dma_start_transpose(
            h2T[:, :, ti * 128:(ti + 1) * 128], h2_bf[:])

    # =============== MLP (data-parallel, bf16 weights streamed) ===========
    for fi in range(F // 128):
        wt = trp.tile([128, 8, 128], BF16, tag="wfc",
                      bufs=6 if trivial else 4)
        dma(out=wt[:], in_=io["w_fc"][fi])
        ps_fc = psum.tile([128, TOK], F32, tag="acc", bufs=3)
        for ci in range(8):
            nc.tensor.matmul(ps_fc[:], wt[:, ci, :], h2T[:, ci, :],
                             start=(ci == 0), stop=(ci == 7))
        if trivial:
            nc.scalar.activation(out=gT[:, fi, :], in_=ps_fc[:],
                                 func=AF.Gelu)
        else:
            nc.scalar.activation(out=gT[:, fi, :], in_=ps_fc[:],
                                 func=AF.Gelu, bias=bfc_sb[:, fi:fi + 1],
                                 scale=1.0)

    # mlp-out staged transposed: ot[tok_p, ti, ci, c_l] = proj[c, tok]; the
    # per-ci xbar covers all 4 token tiles at once and overlaps with the
    # remaining proj columns.
    ot = const.tile([128, NTILE, 8, 128], BF16)
    for ci in range(8):
        wt = big.tile([128, F // 128, 128], BF16, tag="wpj", bufs=2)
        dma(out=wt[:], in_=io["w_proj"][ci])
        ps_pj = psum.tile([128, TOK], F32, tag="acc", bufs=3)
        for fi in range(F // 128):
            nc.tensor.matmul(ps_pj[:], wt[:, fi, :], gT[:, fi, :],
                             start=(fi == 0), stop=(fi == F // 128 - 1))
        pj_sb = work.tile([128, TOK], BF16, tag="pj_sb", bufs=2)
        nc.vector.tensor_copy(out=pj_sb[:], in_=ps_pj[:])
        nc.sync.dma_start_transpose(ot[:, :, ci, :], pj_sb[:])

    for ti in range(NTILE):
        out_t = work.tile([128, C], F32, tag="out_t", bufs=2)
        for ci in range(8):
            nc.vector.tensor_tensor(
                out=out_t[:, ci * 128:(ci + 1) * 128], in0=ot[:, ti, ci, :],
                in1=x2_tiles[ti][:, ci * 128:(ci + 1) * 128],
                op=OP.add)
        dma(out=io["out_x"][ti * 128:(ti + 1) * 128, :], in_=out_t[:])


_NC_CACHE = {}


def _build(trivial, num_devices=NCORES):
    key = ("nc", trivial, num_devices)
    if key in _NC_CACHE:
        return _NC_CACHE[key]
    nc = bass.Bass(trn_type="TRN2", target_bir_lowering=False, debug=False,
                   num_devices=num_devices)
    io = {}

    def inp(name, shape, dt=F32):
        io[name] = nc.dram_tensor(name, list(shape), dt,
                                  kind="ExternalInput").ap()

    def outp(name, shape, dt=F32):
        io[name] = nc.dram_tensor(name, list(shape), dt,
                                  kind="ExternalOutput").ap()

    inp("x_s", (TOK, C))
    inp("wq_s", (C, 128), BF16)
    inp("wk_s", (C, 128), BF16)
    inp("wv_s", (C, 128), BF16)
    inp("w_fc", (F // 128, 128, 8, 128), BF16)
    inp("w_proj", (8, 128, F // 128, 128), BF16)
    if not trivial:
        inp("bq_s", (128, 1)); inp("bk_s", (128, 1)); inp("bv_s", (128, 1))
        inp("ln1_w", (1, C)); inp("ln1_b", (1, C))
        inp("ln2_w", (1, C)); inp("ln2_b", (1, C))
        inp("b_fc_s", (128, F // 128))
        inp("b_proj", (1, C))
    outp("out_x", (TOK, C))
    outp("out_k", (B, HL, T, D), BF16)
    outp("out_v", (B, HL, T, D), BF16)

    with tile.TileContext(nc) as tc:
        with ExitStack() as ctx:
            _emit(nc, tc, io, ctx, trivial)
    _split_multi_waits(nc)
    _NC_CACHE[key] = nc
    return nc


def _is_trivial(bq, bk, bv, ln1_w, ln1_b, ln2_w, ln2_b, b_fc, b_proj):
    return (
        np.all(np.asarray(ln1_w) == 1) and np.all(np.asarray(ln1_b) == 0)
        and np.all(np.asarray(ln2_w) == 1) and np.all(np.asarray(ln2_b) == 0)
        and np.all(np.asarray(bq) == 0) and np.all(np.asarray(bk) == 0)
        and np.all(np.asarray(bv) == 0) and np.all(np.asarray(b_fc) == 0)
        and np.all(np.asarray(b_proj) == 0))


def _make_in_maps(inputs):
    x = np.asarray(inputs["x"], np.float32)
    wq = inputs["wq"]; bq = inputs["bq"]; wk = inputs["wk"]
    bk = inputs["bk"]; wv = inputs["wv"]; bv = inputs["bv"]
    ln1_w = inputs["ln1_w"]; ln1_b = inputs["ln1_b"]
    ln2_w = inputs["ln2_w"]; ln2_b = inputs["ln2_b"]
    w_fc = inputs["w_fc"]; b_fc = inputs["b_fc"]
    w_proj = inputs["w_proj"]; b_proj = inputs["b_proj"]

    xf = np.ascontiguousarray(x.reshape(N, C))
    scale = 1.0 / np.sqrt(D)
    wq_s = np.asarray(wq, np.float32) * scale
    bq_sc = np.asarray(bq, np.float32) * scale
    wk = np.asarray(wk, np.float32); bk = np.asarray(bk, np.float32)
    wv = np.asarray(wv, np.float32); bv = np.asarray(bv, np.float32)
    trivial = _is_trivial(bq, bk, bv, ln1_w, ln1_b, ln2_w, ln2_b,
                          b_fc, b_proj)

    bf = ml_dtypes.bfloat16
    # tiled so each stationary-weight DMA is one contiguous chunk:
    # w_fc  -> [fi, p, ci, h] ; w_proj -> [ci, p, fi, h]
    w_fc_bf = np.ascontiguousarray(
        np.asarray(w_fc, np.float32).astype(bf)
        .reshape(8, 128, F // 128, 128).transpose(2, 1, 0, 3))
    w_proj_bf = np.ascontiguousarray(
        np.asarray(w_proj, np.float32).astype(bf)
        .reshape(F // 128, 128, 8, 128).transpose(2, 1, 0, 3))

    in_maps = []
    for r in range(NCORES):
        ch = slice(r * HL * D, (r + 1) * HL * D)
        m = {
            "x_s": np.ascontiguousarray(xf[r * TOK:(r + 1) * TOK]),
            "wq_s": np.ascontiguousarray(wq_s[:, ch]).astype(bf),
            "wk_s": np.ascontiguousarray(wk[:, ch]).astype(bf),
            "wv_s": np.ascontiguousarray(wv[:, ch]).astype(bf),
            "w_fc": w_fc_bf,
            "w_proj": w_proj_bf,
        }
        if not trivial:
            m.update({
                "bq_s": np.ascontiguousarray(bq_sc[ch]).reshape(128, 1),
                "bk_s": np.ascontiguousarray(bk[ch]).reshape(128, 1),
                "bv_s": np.ascontiguousarray(bv[ch]).reshape(128, 1),
                "ln1_w": np.asarray(ln1_w, np.float32).reshape(1, C),
                "ln1_b": np.asarray(ln1_b, np.float32).reshape(1, C),
                "ln2_w": np.asarray(ln2_w, np.float32).reshape(1, C),
                "ln2_b": np.asarray(ln2_b, np.float32).reshape(1, C),
                "b_fc_s": np.ascontiguousarray(
                    np.asarray(b_fc, np.float32).reshape(F // 128, 128).T),
                "b_proj": np.asarray(b_proj, np.float32).reshape(1, C),
            })
        in_maps.append(m)
    return in_maps, trivial


def kernel(x, wq, bq, wk, bk, wv, bv, ln1_w, ln1_b, ln2_w, ln2_b,
           w_fc, b_fc, w_proj, b_proj):
    inputs = dict(x=x, wq=wq, bq=bq, wk=wk, bk=bk, wv=wv, bv=bv,
                  ln1_w=ln1_w, ln1_b=ln1_b, ln2_w=ln2_w, ln2_b=ln2_b,
                  w_fc=w_fc, b_fc=b_fc, w_proj=w_proj, b_proj=b_proj)
    in_maps, trivial = _make_in_maps(inputs)
    nc = _build(trivial)

    res = run_bass_kernel_spmd(nc, in_maps, core_ids=list(range(NCORES)))

    out_x = np.concatenate([res.results[r]["out_x"] for r in range(NCORES)],
                           axis=0).reshape(B, T, C)
    k = np.concatenate([res.results[r]["out_k"].astype(np.float32)
                        for r in range(NCORES)], axis=1)  # [B, H, T, D]
    v = np.concatenate([res.results[r]["out_v"].astype(np.float32)
                        for r in range(NCORES)], axis=1)
    return out_x, k, v
